# revision 1
# baseline (speedup 1.0000x reference)
"""DiffusionGPT Trainium2 kernel (bf16 redesign).

Data-parallel over batch: 8 batch elements -> 8 NeuronCores.  Activations
feature-major ([feature partitions, token free]) in bf16; weights are
pre-converted to bf16 on the host (half the DMA, no on-chip conversion).
Attention exploits exact causal N-trimming (bf16 matmuls are full rate at
any N), softmax exp is grouped into wide 2-bank PSUM reads, and biases are
applied in the PSUM->SBUF copies (no bias matmuls).

Shapes: B=8, T=1022, S=1024, E=512, H=8 heads, D=64, F=2048, L=4 layers.
"""

import sys

sys.path.insert(0, "/opt/trn_rl_repo")

from contextlib import ExitStack

import numpy as np
import ml_dtypes

import concourse.bass as bass
import concourse.bacc as bacc
import concourse.tile as tile
from concourse import mybir
from concourse.bass_utils import run_bass_kernel_spmd
from concourse.masks import make_identity
from concourse import library_config

F32 = mybir.dt.float32
F32R = mybir.dt.float32r
BF16 = mybir.dt.bfloat16
AF = mybir.ActivationFunctionType
ALU = mybir.AluOpType
BF = ml_dtypes.bfloat16

B = 8
T = 1022
S = 1024
E = 512
H = 8
D = 64
F = 2048
L = 4
NT = E // 128      # 4 feature tiles
NTT = S // 128     # 8 token tiles
LN_EPS = 1e-5
SCALE = 1.0 / 8.0  # 1/sqrt(D)

CHUNKS = ((0, 512), (512, 1024))


def rr(ap):
    return ap.bitcast(F32R)


def build_nc(num_layers=L, do_head=True):
    nc = bacc.Bacc("TRN2", target_bir_lowering=False, debug=False)

    # ---- DRAM I/O ----
    d_sa = nc.dram_tensor("state_actions", [T, 72], F32, kind="ExternalInput")
    d_goals = nc.dram_tensor("goals", [1, 3], F32, kind="ExternalInput")
    d_sigma = nc.dram_tensor("sigma", [1], F32, kind="ExternalInput")
    d_sigma_w = nc.dram_tensor("sigma_w", [1, E], F32, kind="ExternalInput")
    d_sigma_b = nc.dram_tensor("sigma_b", [E], F32, kind="ExternalInput")
    d_tok_w = nc.dram_tensor("tok_w", [72, E], F32, kind="ExternalInput")
    d_tok_b = nc.dram_tensor("tok_b", [E], F32, kind="ExternalInput")
    d_goal_w = nc.dram_tensor("goal_w", [3, E], F32, kind="ExternalInput")
    d_goal_b = nc.dram_tensor("goal_b", [E], F32, kind="ExternalInput")
    d_pos = nc.dram_tensor("pos_emb", [1, S, E], F32, kind="ExternalInput")
    d_ln1_g = nc.dram_tensor("ln1_g", [L, E], F32, kind="ExternalInput")
    d_ln1_b = nc.dram_tensor("ln1_b", [L, E], F32, kind="ExternalInput")
    d_q_b = nc.dram_tensor("q_b", [L, E], F32, kind="ExternalInput")
    d_k_b = nc.dram_tensor("k_b", [L, E], F32, kind="ExternalInput")
    d_v_b = nc.dram_tensor("v_b", [L, E], F32, kind="ExternalInput")
    d_proj_b = nc.dram_tensor("proj_b", [L, E], F32, kind="ExternalInput")
    d_ln2_g = nc.dram_tensor("ln2_g", [L, E], F32, kind="ExternalInput")
    d_ln2_b = nc.dram_tensor("ln2_b", [L, E], F32, kind="ExternalInput")
    d_b1 = nc.dram_tensor("mlp_b1", [L, F], F32, kind="ExternalInput")
    d_b2 = nc.dram_tensor("mlp_b2", [L, E], F32, kind="ExternalInput")
    d_lnf_g = nc.dram_tensor("lnf_g", [E], F32, kind="ExternalInput")
    d_lnf_b = nc.dram_tensor("lnf_b", [E], F32, kind="ExternalInput")
    d_pred_b = nc.dram_tensor("pred_b", [72], F32, kind="ExternalInput")
    # host-converted bf16 weights
    d_q_w = nc.dram_tensor("q_w16", [L, E, E], BF16, kind="ExternalInput")
    d_k_w = nc.dram_tensor("k_w16", [L, E, E], BF16, kind="ExternalInput")
    d_v_w = nc.dram_tensor("v_w16", [L, E, E], BF16, kind="ExternalInput")
    d_proj_w = nc.dram_tensor("proj_w16", [L, E, E], BF16, kind="ExternalInput")
    d_w1 = nc.dram_tensor("mlp_w116", [L, E, F], BF16, kind="ExternalInput")
    d_w2 = nc.dram_tensor("mlp_w216", [L, F, E], BF16, kind="ExternalInput")
    d_pred_w = nc.dram_tensor("pred_w16", [E, 72], BF16, kind="ExternalInput")
    d_out = nc.dram_tensor("out", [T, 72], F32, kind="ExternalOutput")

    with tile.TileContext(nc) as tc, ExitStack() as ctx:
        nc.gpsimd.load_library(library_config.attnmlp)

        const = ctx.enter_context(tc.tile_pool(name="const", bufs=1))
        big = ctx.enter_context(tc.tile_pool(name="big", bufs=1))
        wqkv = ctx.enter_context(tc.tile_pool(name="wqkv", bufs=32))
        w1p = ctx.enter_context(tc.tile_pool(name="w1p", bufs=6))
        w2p = ctx.enter_context(tc.tile_pool(name="w2p", bufs=18))
        bmat = ctx.enter_context(tc.tile_pool(name="bmat", bufs=2))
        bcols = ctx.enter_context(tc.tile_pool(name="bcols", bufs=8))
        ptp = ctx.enter_context(tc.tile_pool(name="ptp", bufs=6))
        usp = ctx.enter_context(tc.tile_pool(name="usp", bufs=16))
        rowp = ctx.enter_context(tc.tile_pool(name="rowp", bufs=5))
        bbp = ctx.enter_context(tc.tile_pool(name="bbp", bufs=3))
        recp = ctx.enter_context(tc.tile_pool(name="recp", bufs=2))
        scr = ctx.enter_context(tc.tile_pool(name="scr", bufs=4))

        # PSUM (8 banks): sc 2x2 + b 2 + o 1 + tpb 1
        ps_sc = ctx.enter_context(tc.tile_pool(name="ps_sc", bufs=2, space="PSUM"))
        ps_b = ctx.enter_context(tc.tile_pool(name="ps_b", bufs=2, space="PSUM"))
        ps_o = ctx.enter_context(tc.tile_pool(name="ps_o", bufs=1, space="PSUM"))
        ps_st = ps_b

        # ---- constants ----
        ident = const.tile([128, 128], F32)
        make_identity(nc, ident[:])
        ident16 = const.tile([128, 128], BF16)
        nc.vector.tensor_copy(ident16[:], ident[:])
        ones64_16 = const.tile([128, 64], BF16)
        nc.gpsimd.memset(ones64_16[:], 1.0)
        ones_col16 = ones64_16[:, 0:1]
        eps_col = const.tile([128, 1], F32)
        nc.gpsimd.memset(eps_col[:], LN_EPS)

        # persistent activations (feature-major bf16)
        x_t = [big.tile([128, S], BF16, name=f"x{i}") for i in range(NT)]
        h_t = [big.tile([128, S], BF16, name=f"h{i}") for i in range(NT)]
        q_t = [big.tile([128, S], BF16, name=f"qa{i}") for i in range(NT)]
        k_t = [big.tile([128, S], BF16, name=f"ka{i}") for i in range(NT)]
        v_t = [big.tile([128, S], BF16, name=f"va{i}") for i in range(NT)]
        y_t = h_t
        sq_t = [big.tile([128, 512], BF16, name=f"sq{i}") for i in range(NT)]
        # v token-major with ones column per head: [128 tok, 8 heads x 65]
        vtok = [big.tile([128, H * 65], BF16, name=f"vtok{i}") for i in range(NTT)]
        for kt in range(NTT):
            vt3 = vtok[kt].rearrange("p (h c) -> p h c", c=65)
            nc.vector.tensor_copy(
                vt3[:, :, 64:65],
                ones64_16.rearrange("p (b c) -> p b c", c=1)[:, 0:H, :])

        # =================================================================
        # Embedding (f32 path, writes bf16 x)
        # =================================================================
        ones16_row = const.tile([1, 1024], BF16)
        nc.gpsimd.memset(ones16_row[:], 1.0)
        saT = const.tile([73, T], BF16)
        # engines can't start at partition 72; DMA can write any partition
        nc.sync.dma_start(saT[72:73, :], ones16_row[:, 0:T])
        for tt in range(NTT):
            ntt = min(128, T - tt * 128)
            sa_tok = scr.tile([128, 72], F32, tag="sa_tok")
            nc.sync.dma_start(sa_tok[0:ntt, :], d_sa[tt * 128: tt * 128 + ntt, :])
            sa16 = scr.tile([128, 72], BF16, tag="sa16", bufs=2)
            nc.vector.tensor_copy(sa16[0:ntt, :], sa_tok[0:ntt, :])
            tp = ps_o.tile([128, 128], BF16, tag="tpb", bufs=1)
            nc.tensor.matmul(tp[0:72, 0:ntt], sa16[0:ntt, 0:72],
                             ident16[0:ntt, 0:ntt], is_transpose=True)
            nc.vector.tensor_copy(saT[0:72, tt * 128: tt * 128 + ntt], tp[0:72, 0:ntt])

        tokw_f = const.tile([73, E], F32)
        nc.sync.dma_start(tokw_f[0:72, :], d_tok_w[:, :])
        nc.sync.dma_start(tokw_f[72:73, :], d_tok_b.ap().rearrange("(a e) -> a e", a=1))
        tokw_aug = const.tile([73, E], BF16)
        nc.vector.tensor_copy(tokw_aug[:], tokw_f[:])

        # sigma & goal columns: K=7 matmul per feature tile
        G_sf = const.tile([7, E], F32)
        nc.sync.dma_start(G_sf[0:1, :], d_sigma_w[:, :])
        nc.sync.dma_start(G_sf[1:2, :], d_sigma_b.ap().rearrange("(a e) -> a e", a=1))
        nc.sync.dma_start(G_sf[2:5, :], d_goal_w[:, :])
        nc.sync.dma_start(G_sf[5:6, :], d_goal_b.ap().rearrange("(a e) -> a e", a=1))
        nc.sync.dma_start(G_sf[6:7, :], d_pos.ap()[0, 0:1, :])
        G_sb = const.tile([7, E], BF16)
        nc.vector.tensor_copy(G_sb[:], G_sf[:])

        sig_sb = const.tile([1, 1], F32)
        nc.sync.dma_start(sig_sb[:], d_sigma.ap().rearrange("(a e) -> a e", a=1))
        lns = const.tile([1, 1], F32)
        nc.scalar.activation(lns[:], sig_sb[:], AF.Ln)
        sg_rowf = const.tile([1, 14], F32)
        nc.gpsimd.memset(sg_rowf[:], 0.0)
        nc.scalar.activation(sg_rowf[0:1, 0:1], lns[:], AF.Copy, scale=0.25)
        nc.gpsimd.memset(sg_rowf[0:1, 1:2], 1.0)
        g_row = const.tile([1, 3], F32)
        nc.sync.dma_start(g_row[:], d_goals[:, :])
        nc.vector.tensor_copy(sg_rowf[0:1, 9:12], g_row[:])
        nc.gpsimd.memset(sg_rowf[0:1, 12:14], 1.0)
        sg_row = const.tile([1, 14], BF16)
        nc.vector.tensor_copy(sg_row[:], sg_rowf[:])
        sg_rhs = const.tile([7, 2], BF16)
        for col in range(2):
            gtp = ps_o.tile([128, 128], BF16, tag="tpb", bufs=1)
            nc.tensor.matmul(gtp[0:7, 0:1], sg_row[0:1, col * 7:(col + 1) * 7],
                             ident16[0:1, 0:1], is_transpose=True)
            nc.vector.tensor_copy(sg_rhs[:, col: col + 1], gtp[0:7, 0:1])

        for fc in range(NT):
            sg_ps = ps_o.tile([128, 512], F32, tag="o")
            nc.tensor.matmul(sg_ps[0:128, 0:2], G_sb[:, fc * 128:(fc + 1) * 128],
                             sg_rhs[:], start=True, stop=True)
            nc.scalar.activation(x_t[fc][:, 0:2], sg_ps[0:128, 0:2], AF.Copy)

        # sa tokens -> x columns 2..1023
        for tt in range(NTT):
            ntt = min(128, T - tt * 128)
            xe_ps = ps_b.tile([128, 512], F32, tag="b")
            nc.tensor.matmul(xe_ps[0:ntt, :], saT[:, tt * 128: tt * 128 + ntt],
                             tokw_aug[:], start=True, stop=True)
            pos_sb = ptp.tile([128, E], F32, tag="pT", bufs=3)
            nc.sync.dma_start(pos_sb[0:ntt, :],
                              d_pos.ap()[0, tt * 128 + 1: tt * 128 + 1 + ntt, :])
            xe_tok = ptp.tile([128, E], BF16, tag="pT16", bufs=3)
            nc.vector.tensor_add(xe_tok[0:ntt, :], xe_ps[0:ntt, :], pos_sb[0:ntt, :])
            for fc in range(NT):
                tp = ps_o.tile([128, 128], BF16, tag="tpb", bufs=1)
                nc.tensor.matmul(tp[:, 0:ntt],
                                 xe_tok[0:ntt, fc * 128:(fc + 1) * 128],
                                 ident16[0:ntt, 0:ntt], is_transpose=True)
                nc.vector.tensor_copy(
                    x_t[fc][:, 2 + tt * 128: 2 + tt * 128 + ntt], tp[:, 0:ntt])

        # =================================================================
        # helpers
        # =================================================================
        def ln_chunk(c, src_t, g_cols, b_cols, dst_t):
            """LayerNorm of chunk c: src (bf16) -> dst (bf16)."""
            c0, c1 = CHUNKS[c]
            for ti in range(NT):
                nc.vector.tensor_mul(sq_t[ti][:], src_t[ti][:, c0:c1],
                                     src_t[ti][:, c0:c1])
            s1t = ps_st.tile([128, 512], F32, tag="b")
            s1 = s1t[0:1, :]
            for ti in range(NT):
                nc.tensor.matmul(s1, ones_col16, src_t[ti][:, c0:c1],
                                 start=(ti == 0), stop=(ti == NT - 1))
            mean_row = rowp.tile([1, 512], F32, tag="rows")
            nc.vector.tensor_scalar_mul(mean_row[:], s1, 1.0 / E)
            s2t = ps_st.tile([128, 512], F32, tag="b")
            s2 = s2t[0:1, :]
            for ti in range(NT):
                nc.tensor.matmul(s2, ones_col16, sq_t[ti][:],
                                 start=(ti == 0), stop=(ti == NT - 1))
            m2_row = rowp.tile([1, 512], F32, tag="rows")
            nc.vector.tensor_scalar(m2_row[:], s2, 1.0 / E, LN_EPS,
                                    ALU.mult, ALU.add)
            msq = rowp.tile([1, 512], F32, tag="rows")
            nc.vector.tensor_mul(msq[:], mean_row[:], mean_row[:])
            nc.vector.tensor_sub(m2_row[:], m2_row[:], msq[:])
            w_row = rowp.tile([1, 512], F32, tag="rows")
            nc.vector.reciprocal(w_row[:], m2_row[:])
            rstd_row = rowp.tile([1, 512], F32, tag="rows")
            nc.scalar.activation(rstd_row[:], w_row[:], AF.Sqrt)
            mb = bbp.tile([128, 512], F32, tag="bb")
            nc.gpsimd.partition_broadcast(mb[:], mean_row[:])
            rb = bbp.tile([128, 512], F32, tag="bb")
            nc.gpsimd.partition_broadcast(rb[:], rstd_row[:])
            for ti in range(NT):
                t0 = sq_t[ti]  # reuse square scratch as LN scratch
                nc.vector.tensor_sub(t0[:], src_t[ti][:, c0:c1], mb[:])
                nc.vector.tensor_mul(t0[:], t0[:], rb[:])
                nc.vector.tensor_scalar(dst_t[ti][:, c0:c1], t0[:],
                                        g_cols[ti], b_cols[ti],
                                        ALU.mult, ALU.add)

        def matmul_out(c, w_sb, in_t):
            """yield (ot, psum tile) for out = w^T @ in over chunk c.
            Alternates between the o/b PSUM rings for depth-2 pipelining."""
            c0, c1 = CHUNKS[c]
            for ot in range(NT):
                if ot % 2 == 0:
                    ps = ps_o.tile([128, 512], F32, tag="o")
                else:
                    ps = ps_b.tile([128, 512], F32, tag="b")
                for kc in range(NT):
                    nc.tensor.matmul(
                        ps[:], w_sb[kc][:, ot * 128:(ot + 1) * 128],
                        in_t[kc][:, c0:c1],
                        start=(kc == 0), stop=(kc == NT - 1))
                yield ot, ps

        # =================================================================
        # Transformer layers
        # =================================================================
        wq_sb = wk_sb = wv_sb = wp_sb = None
        w1_sb = w2_sb = None

        def load_qkvp(l):
            res = []
            for dw in (d_q_w, d_k_w, d_v_w, d_proj_w):
                tiles = []
                for kc in range(NT):
                    wt = wqkv.tile([128, E], BF16, tag="w")
                    nc.sync.dma_start(wt[:], dw.ap()[l, kc * 128:(kc + 1) * 128, :])
                    tiles.append(wt)
                res.append(tiles)
            return res

        def load_w1(l):
            tiles = []
            for kc in range(NT):
                wt = w1p.tile([128, F], BF16, tag="w1")
                nc.sync.dma_start(wt[:], d_w1.ap()[l, kc * 128:(kc + 1) * 128, :])
                tiles.append(wt)
            return tiles

        def load_w2(l):
            tiles = []
            for h16 in range(F // 128):
                wt = w2p.tile([128, E], BF16, tag="w2")
                nc.sync.dma_start(wt[:], d_w2.ap()[l, h16 * 128:(h16 + 1) * 128, :])
                tiles.append(wt)
            return tiles

        # prefetch layer 0 weights
        wq_sb, wk_sb, wv_sb, wp_sb = load_qkvp(0)
        w1_sb = load_w1(0)
        w2_sb = load_w2(0)

        for l in range(num_layers):
            # ---- per-layer bias/gain columns ----
            # rows: 0 ln1_g, 1 ln1_b, 2 ln2_g, 3 ln2_b, 4 q_b, 5 k_b, 6 v_b,
            #       7 proj_b, 8 mlp_b2, 9..12 mlp_b1
            Bm = bmat.tile([13, E], F32, tag="B")
            nc.sync.dma_start(Bm[0:1, :], d_ln1_g.ap()[l: l + 1, :])
            nc.sync.dma_start(Bm[1:2, :], d_ln1_b.ap()[l: l + 1, :])
            nc.sync.dma_start(Bm[2:3, :], d_ln2_g.ap()[l: l + 1, :])
            nc.sync.dma_start(Bm[3:4, :], d_ln2_b.ap()[l: l + 1, :])
            nc.sync.dma_start(Bm[4:5, :], d_q_b.ap()[l: l + 1, :])
            nc.sync.dma_start(Bm[5:6, :], d_k_b.ap()[l: l + 1, :])
            nc.sync.dma_start(Bm[6:7, :], d_v_b.ap()[l: l + 1, :])
            nc.sync.dma_start(Bm[7:8, :], d_proj_b.ap()[l: l + 1, :])
            nc.sync.dma_start(Bm[8:9, :], d_b2.ap()[l: l + 1, :])
            nc.sync.dma_start(Bm[9:13, :],
                              d_b1.ap()[l: l + 1, :].rearrange("a (b e) -> (a b) e", e=E))
            Bm16 = bmat.tile([13, E], BF16, tag="B16")
            nc.vector.tensor_copy(Bm16[:], Bm[:])
            bc_t = []
            for fc in range(NT):
                tp = ps_o.tile([128, 128], BF16, tag="tpb", bufs=1)
                nc.tensor.matmul(tp[:, 0:13], Bm16[:, fc * 128:(fc + 1) * 128],
                                 ident16[0:13, 0:13], is_transpose=True)
                bct = bcols.tile([128, 13], F32, tag="bc")
                nc.vector.tensor_copy(bct[:], tp[:, 0:13])
                bc_t.append(bct)

            # prefetch next layer's MLP weights (DMAs flow during LN1/QKV/attn)
            if l + 1 < num_layers:
                nw1 = load_w1(l + 1)
                nw2 = load_w2(l + 1)

            g1 = [bc_t[ti][:, 0:1] for ti in range(NT)]
            b1_ = [bc_t[ti][:, 1:2] for ti in range(NT)]
            g2 = [bc_t[ti][:, 2:3] for ti in range(NT)]
            b2_ = [bc_t[ti][:, 3:4] for ti in range(NT)]

            # ---- LN1 + QKV + vtok, both chunks ----
            for c in range(2):
                c0, c1 = CHUNKS[c]
                ln_chunk(c, x_t, g1, b1_, h_t)
                for bidx, w_sb, dst in ((4, wq_sb, q_t), (5, wk_sb, k_t),
                                        (6, wv_sb, v_t)):
                    for ot, ps in matmul_out(c, w_sb, h_t):
                        nc.scalar.activation(
                            dst[ot][:, c0:c1], ps[:], AF.Identity,
                            bias=bc_t[ot][:, bidx:bidx + 1])
                # v -> token-major vtok for this chunk's key tiles
                for kt in range(4 * c, 4 * c + 4):
                    for fc in range(NT):
                        tp = ps_o.tile([128, 128], BF16, tag="tpb", bufs=1)
                        nc.tensor.matmul(tp[:], v_t[fc][:, kt * 128:(kt + 1) * 128],
                                         ident16[:], is_transpose=True)
                        dst = vtok[kt][:, 130 * fc: 130 * fc + 130] \
                            .rearrange("p (h c) -> p h c", c=65)[:, :, 0:64]
                        nc.vector.tensor_copy(
                            dst, tp[:].rearrange("p (h c) -> p h c", c=64))

            # ---- attention, per chunk (queries c0:c1) ----
            for c in range(2):
                c0, c1 = CHUNKS[c]
                # kt groups packed into 2-bank wide PSUM tiles:
                # each group: list of (kt, col, qlo, N, diag)
                if c == 0:
                    groups = [[(0, 0, 0, 512, True), (1, 512, 128, 384, True)],
                              [(2, 0, 256, 256, True), (3, 256, 384, 128, True)]]
                else:
                    groups = [[(0, 0, 0, 512, False), (1, 512, 0, 512, False)],
                              [(2, 0, 0, 512, False), (3, 512, 0, 512, False)],
                              [(4, 0, 0, 512, True), (5, 512, 128, 384, True)],
                              [(6, 0, 256, 256, True), (7, 256, 384, 128, True)]]
                for hd in range(H):
                    ht = hd // 2
                    hp = (hd % 2) * 64
                    q_h = q_t[ht][hp: hp + 64, :]
                    k_h = k_t[ht][hp: hp + 64, :]
                    y_pst = ps_b.tile([128, 512], F32, tag="b")
                    y_ps = y_pst[0:65, :]
                    first = True
                    for grp in groups:
                        wide = ps_sc.tile([128, 1024], F32, tag="sc")
                        gw = max(col + n for (_, col, _, n, _) in grp)
                        for (kt, col, qlo, n, diag) in grp:
                            nc.tensor.matmul(
                                wide[:, col:col + n],
                                k_h[:, kt * 128:(kt + 1) * 128],
                                q_h[:, c0 + qlo:c1],
                                start=True, stop=True)
                        pt = ptp.tile([128, 1024], BF16, tag="pt", bufs=4)
                        nc.scalar.activation(pt[:, 0:gw], wide[:, 0:gw],
                                             AF.Exp, scale=SCALE)
                        for (kt, col, qlo, n, diag) in grp:
                            if diag:
                                nc.gpsimd.affine_select(
                                    out=pt[:, col:col + 128],
                                    in_=pt[:, col:col + 128],
                                    compare_op=ALU.is_ge, fill=0.0,
                                    base=0, pattern=[[1, 128]],
                                    channel_multiplier=-1)
                        for (kt, col, qlo, n, diag) in grp:
                            nc.tensor.matmul(
                                y_ps[:, qlo:512],
                                vtok[kt][:, 65 * hd: 65 * hd + 65],
                                pt[:, col:col + n],
                                start=first, stop=(kt == grp[-1][0]
                                                   and grp is groups[-1]))
                            first = False
                    rec = recp.tile([1, 512], F32, tag="rr", bufs=1)
                    nc.vector.reciprocal(rec[:], y_ps[64:65, :])
                    rec_b = recp.tile([64, 512], F32, tag="rb", bufs=1)
                    nc.gpsimd.partition_broadcast(rec_b[:], rec[:])
                    nc.vector.tensor_mul(y_t[ht][hp: hp + 64, c0:c1],
                                         y_ps[0:64, :], rec_b[:])

            # prefetch next layer's QKV/proj while attention/proj runs
            if l + 1 < num_layers:
                nwq, nwk, nwv, nwp = load_qkvp(l + 1)

            # ---- proj + residual, per chunk ----
            for c in range(2):
                c0, c1 = CHUNKS[c]
                for ot, ps in matmul_out(c, wp_sb, y_t):
                    tmp = scr.tile([128, 512], BF16, tag="tmp")
                    nc.scalar.activation(tmp[:], ps[:], AF.Identity,
                                         bias=bc_t[ot][:, 7:8])
                    nc.vector.tensor_add(x_t[ot][:, c0:c1], x_t[ot][:, c0:c1],
                                         tmp[:])

            # ---- LN2 both chunks, then MLP both chunks ----
            for c in range(2):
                ln_chunk(c, x_t, g2, b2_, h_t)
            for c in range(2):
                c0, c1 = CHUNKS[c]
                u_s = []
                for h16 in range(F // 128):
                    u_ps = ps_b.tile([128, 512], F32, tag="b")
                    for kc in range(NT):
                        nc.tensor.matmul(
                            u_ps[:], w1_sb[kc][:, h16 * 128:(h16 + 1) * 128],
                            h_t[kc][:, c0:c1],
                            start=(kc == 0), stop=(kc == NT - 1))
                    us = usp.tile([128, 512], BF16, tag="us")
                    b1col = bc_t[h16 % 4][:, 9 + h16 // 4: 10 + h16 // 4]
                    nc.scalar.activation(us[:], u_ps[:], AF.Gelu, bias=b1col)
                    u_s.append(us)
                for ot in range(NT):
                    ps = ps_o.tile([128, 512], F32, tag="o")
                    for h16 in range(F // 128):
                        nc.tensor.matmul(
                            ps[:], w2_sb[h16][:, ot * 128:(ot + 1) * 128],
                            u_s[h16][:], start=(h16 == 0), stop=(h16 == 15))
                    tmp = scr.tile([128, 512], BF16, tag="tmp")
                    nc.scalar.activation(tmp[:], ps[:], AF.Identity,
                                         bias=bc_t[ot][:, 8:9])
                    nc.vector.tensor_add(x_t[ot][:, c0:c1], x_t[ot][:, c0:c1],
                                         tmp[:])
            if l + 1 < num_layers:
                wq_sb, wk_sb, wv_sb, wp_sb = nwq, nwk, nwv, nwp
                w1_sb, w2_sb = nw1, nw2

        # =================================================================
        # Final LN + prediction head + output transpose
        # =================================================================
        if do_head:
            B2 = bmat.tile([13, E], F32, tag="B")
            nc.sync.dma_start(B2[0:1, :], d_lnf_g.ap().rearrange("(a e) -> a e", a=1))
            nc.sync.dma_start(B2[1:2, :], d_lnf_b.ap().rearrange("(a e) -> a e", a=1))
            B216 = bmat.tile([13, E], BF16, tag="B16")
            nc.vector.tensor_copy(B216[0:2, :], B2[0:2, :])
            bcf_t = []
            for fc in range(NT):
                tp = ps_o.tile([128, 128], BF16, tag="tpb", bufs=1)
                nc.tensor.matmul(tp[:, 0:2], B216[0:2, fc * 128:(fc + 1) * 128],
                                 ident16[0:2, 0:2], is_transpose=True)
                bct = bcols.tile([128, 13], F32, tag="bc")
                nc.vector.tensor_copy(bct[:, 0:2], tp[:, 0:2])
                bcf_t.append(bct)
            gf = [bcf_t[ti][:, 0:1] for ti in range(NT)]
            bf_ = [bcf_t[ti][:, 1:2] for ti in range(NT)]
            pw_sb = []
            for kc in range(NT):
                wt = wqkv.tile([128, 72], BF16, tag="pw", bufs=4)
                nc.sync.dma_start(wt[:], d_pred_w.ap()[kc * 128:(kc + 1) * 128, :])
                pw_sb.append(wt)
            pb_rowf = const.tile([1, 72], F32)
            nc.sync.dma_start(pb_rowf[:], d_pred_b.ap().rearrange("(a e) -> a e", a=1))
            pb_row = const.tile([1, 72], BF16)
            nc.vector.tensor_copy(pb_row[:], pb_rowf[:])
            pb_col = const.tile([72, 1], F32)
            ptps = ps_o.tile([128, 128], BF16, tag="tpb", bufs=1)
            nc.tensor.matmul(ptps[0:72, 0:1], pb_row[:], ident16[0:1, 0:1],
                             is_transpose=True)
            nc.vector.tensor_copy(pb_col[:], ptps[0:72, 0:1])

            outT = const.tile([72, T], BF16)

            for c in range(2):
                # pred token range aligned to LN chunk: [2:512) / [512:1024)
                ln_chunk(c, x_t, gf, bf_, h_t)
                c0 = 2 if c == 0 else 512
                c1 = 512 if c == 0 else S
                n = c1 - c0
                ps = ps_o.tile([128, 512], F32, tag="o")
                for kc in range(NT):
                    nc.tensor.matmul(ps[0:72, 0:n], pw_sb[kc][:],
                                     h_t[kc][:, c0:c1], start=(kc == 0),
                                     stop=(kc == NT - 1))
                nc.scalar.activation(outT[:, c0 - 2: c1 - 2], ps[0:72, 0:n],
                                     AF.Identity, bias=pb_col[:, 0:1])

            for tt in range(NTT):
                ntt = min(128, T - tt * 128)
                tp = ps_o.tile([128, 128], BF16, tag="tpb", bufs=1)
                nc.tensor.matmul(tp[0:ntt, 0:72], outT[:, tt * 128: tt * 128 + ntt],
                                 ident16[0:72, 0:72], is_transpose=True)
                o_sb = scr.tile([128, 72], F32, tag="sa_tok")
                nc.vector.tensor_copy(o_sb[0:ntt, :], tp[0:ntt, 0:72])
                nc.sync.dma_start(d_out.ap()[tt * 128: tt * 128 + ntt, :],
                                  o_sb[0:ntt, :])

    nc.compile()
    return nc


_NC_CACHE = None


def _get_nc():
    global _NC_CACHE
    if _NC_CACHE is None:
        _NC_CACHE = build_nc()
    return _NC_CACHE


F32_WEIGHTS = [
    "sigma_w", "sigma_b", "tok_w", "tok_b", "goal_w", "goal_b", "pos_emb",
    "ln1_g", "ln1_b", "q_b", "k_b", "v_b", "proj_b", "ln2_g", "ln2_b",
    "mlp_b1", "mlp_b2", "lnf_g", "lnf_b", "pred_b",
]
BF16_WEIGHTS = ["q_w", "k_w", "v_w", "proj_w", "mlp_w1", "mlp_w2", "pred_w"]


def make_in_maps(inputs):
    sa = np.asarray(inputs["state_actions"], np.float32)
    goals = np.asarray(inputs["goals"], np.float32)
    sigma = np.asarray(inputs["sigma"], np.float32)
    shared = {n: np.ascontiguousarray(np.asarray(inputs[n], np.float32))
              for n in F32_WEIGHTS}
    for n in BF16_WEIGHTS:
        shared[n + "16"] = np.ascontiguousarray(
            np.asarray(np.asarray(inputs[n], np.float32), BF))
    in_maps = []
    for b in range(B):
        m = dict(shared)
        m["state_actions"] = np.ascontiguousarray(sa[b])
        m["goals"] = np.ascontiguousarray(goals[b])
        m["sigma"] = np.ascontiguousarray(sigma[b: b + 1])
        in_maps.append(m)
    return in_maps


def run_spmd(inputs, **kwargs):
    nc = _get_nc()
    res = run_bass_kernel_spmd(nc, make_in_maps(inputs), list(range(B)), **kwargs)
    out = np.stack([res.results[c]["out"] for c in range(B)], axis=0)
    return out.astype(np.float32), res


def kernel(**inputs):
    out, _ = run_spmd(inputs)
    return out

